# revision 2
# baseline (speedup 1.0000x reference)
"""Single-head attention (B=8, S=2048, D=1024, H=64) on 8 TRN2 NeuronCores.

Sharding: data-parallel over batch — one batch element per core, Q/K/V
weights replicated. No collectives; host gathers the 8 per-core outputs.

Per-core pipeline (all shapes per core):
  phase 1: x [S, D] -> PE-transpose 128x128 tiles -> xT (bf16)
           fused QKV matmul (xT stationary, W^T moving, N=192) -> q|k|v
           + bias add (DVE), q/k PE-transposed into qT/kT [H, S] (bf16),
           v kept natural in v_aug [S, H+1] with a ones column.
  phase 2: scores = qT.T @ kT per 128-row q-tile (PSUM f32)
           masked = (scores + C) * mask   (one fused DVE op, f32)
           PE-transpose masked -> [k, q] layout (f32r, exact bit movement)
           probsT = exp(0.125*x - C/8)    (ACT, psum->sbuf, bf16 out)
           outT[65, q] += v_aug[k-tile].T @ probsT  (PSUM accum over k)
           row 64 = sum of exps -> PE-transpose back, normalize by
           reciprocal (DVE), DMA out.

The +C shift keeps masked scores exactly (s+C)*m: m==1 -> s after the
exp bias -C/8 removes the shift; m==0 -> exp(-C/8) = e^-37.5 ~ 5e-17.
"""

import sys
import types

import numpy as np

import concourse.bass as bass
import concourse.mybir as mybir
import concourse.tile as tile
from concourse import bacc
from concourse.bass_utils import run_bass_kernel_spmd
from concourse.masks import make_identity

B, S, D, H = 8, 2048, 1024, 64
NT = S // 128          # 16 seq tiles of 128
NCH = D // 128         # 8 contraction chunks
NG = 4                 # q-tile groups of 4 (512 q columns per group)
C_SHIFT = 300.0

f32 = mybir.dt.float32
f32r = mybir.dt.float32r
bf16 = mybir.dt.bfloat16
i32 = mybir.dt.int32
ALU = mybir.AluOpType
ACT_EXP = mybir.ActivationFunctionType.Exp


def install_ntff_hook():
    """RL-container antenv stub lacks axon_hooks; inject it so trace=True
    under axon can capture NTFF profiles. Harmless if already present."""
    if "antenv.axon_hooks" in sys.modules:
        return
    try:
        mod = types.ModuleType("antenv.axon_hooks")
        state = {"hook": None}
        mod.set_axon_ntff_profile_hook = lambda h: state.__setitem__("hook", h)
        mod.get_axon_ntff_profile_hook = lambda: state["hook"]
        sys.modules["antenv.axon_hooks"] = mod
        import antenv

        antenv.axon_hooks = mod
        from trn_agent_boot.trn_boot import _ntff_profile_via_ctypes

        mod.set_axon_ntff_profile_hook(
            _ntff_profile_via_ctypes("/opt/axon/libaxon_pjrt.so")
        )
    except Exception:
        pass


def build():
    nc = bacc.Bacc("TRN2", target_bir_lowering=False, debug=False, num_devices=8)

    x_d = nc.dram_tensor("input", [S, D], f32, kind="ExternalInput")
    m_d = nc.dram_tensor("mask", [S, S], i32, kind="ExternalInput")
    w_d = {
        n: nc.dram_tensor(n, [H, D], f32, kind="ExternalInput")
        for n in ("W_q", "W_k", "W_v")
    }
    b_d = {
        n: nc.dram_tensor(n, [H], f32, kind="ExternalInput")
        for n in ("b_q", "b_k", "b_v")
    }
    out_d = nc.dram_tensor("out", [S, H], f32, kind="ExternalOutput")

    with tile.TileContext(nc) as tc:
        with (
            tc.tile_pool(name="singles", bufs=1) as singles,
            tc.tile_pool(name="sb", bufs=2) as sb,
            tc.tile_pool(name="msk", bufs=6) as mskp,
            tc.tile_pool(name="mkin", bufs=3) as mkin,
            tc.tile_pool(name="pA", bufs=2, space="PSUM") as pA,
            tc.tile_pool(name="pB", bufs=2, space="PSUM") as pB,
            tc.tile_pool(name="pPV", bufs=2, space="PSUM") as pPV,
        ):
            # ---- constants -------------------------------------------------
            ident = singles.tile([128, 128], f32)
            make_identity(nc, ident[:])
            id_r = ident[:]
            id_b = singles.tile([128, 128], bf16)
            make_identity(nc, id_b[:])

            exp_bias = singles.tile([128, 1], f32)
            nc.gpsimd.memset(exp_bias[:], -C_SHIFT / 8.0)

            bias_bc = singles.tile([128, 192], f32)
            for wi, n in enumerate(("b_q", "b_k", "b_v")):
                src = bass.AP(tensor=b_d[n], offset=0, ap=[[0, 128], [1, H]])
                nc.gpsimd.dma_start(bias_bc[:, wi * H:(wi + 1) * H], src)

            # ---- weights: W^T in bf16, laid out [128, chunk, q|k|v] -------
            wT = singles.tile([128, NCH, 192], bf16)
            for wi, n in enumerate(("W_q", "W_k", "W_v")):
                w_nat = sb.tile([H, D], f32, tag="wnat")
                nc.sync.dma_start(w_nat[:], w_d[n].ap())
                for c in range(NCH):
                    wt_ps = pA.tile([128, H], f32, tag="A")
                    nc.tensor.transpose(
                        wt_ps[:],
                        w_nat[:, c * 128:(c + 1) * 128],
                        id_r[:H, :H],
                    )
                    nc.scalar.copy(
                        wT[:, c, wi * H:(wi + 1) * H], wt_ps[:]
                    )

            # persistent activations
            qT = singles.tile([H, S], bf16)
            kT = singles.tile([H, S], bf16)
            v_aug = singles.tile([128, NT, H + 1], bf16)
            nc.gpsimd.memset(v_aug[:, :, H:H + 1], 1.0)

            # ---- phase 1: project ------------------------------------------
            for t in range(NT):
                x_t = sb.tile([128, D], f32, tag="x")
                nc.sync.dma_start(x_t[:], x_d.ap()[t * 128:(t + 1) * 128, :])

                xt_ps = pB.tile([128, D], f32, tag="B")
                for c in range(NCH):
                    nc.tensor.transpose(
                        xt_ps[:, c * 128:(c + 1) * 128],
                        x_t[:, c * 128:(c + 1) * 128],
                        id_r,
                    )
                xT_sb = sb.tile([128, NCH, 128], bf16, tag="xT")
                cp = nc.scalar.copy if t % 2 == 0 else (
                    lambda o, i: nc.vector.tensor_copy(o, i)
                )
                cp(
                    xT_sb[:].rearrange("p c f -> p (c f)"),
                    xt_ps[:],
                )

                pj_ps = pA.tile([128, 192], f32, tag="A")
                for c in range(NCH):
                    nc.tensor.matmul(
                        pj_ps[:],
                        xT_sb[:, c, :],
                        wT[:, c, :],
                        start=(c == 0),
                        stop=(c == NCH - 1),
                    )
                qkv_sb = sb.tile([128, 192], bf16, tag="qkv")
                nc.vector.tensor_add(qkv_sb[:], pj_ps[:], bias_bc[:])

                nc.scalar.copy(v_aug[:, t, 0:H], qkv_sb[:, 128:192])

                for which, dst in ((0, qT), (1, kT)):
                    tp = pA.tile([H, 128], bf16, tag="A")
                    nc.tensor.transpose(
                        tp[:], qkv_sb[:, which * H:(which + 1) * H], id_b[:]
                    )
                    nc.scalar.copy(dst[:, t * 128:(t + 1) * 128], tp[:])

            # ---- phase 2: attention ----------------------------------------
            for g in range(NG):
                masked_g = []
                for qq in range(4):
                    qt = g * 4 + qq
                    mask_t = mkin.tile([128, S], i32, tag="mk")
                    nc.sync.dma_start(
                        mask_t[:], m_d.ap()[qt * 128:(qt + 1) * 128, :]
                    )
                    masked_t = mskp.tile([128, S], f32, tag="msk")
                    for ch in range(4):
                        sl = slice(ch * 512, (ch + 1) * 512)
                        sc_ps = pA.tile([128, 512], f32, tag="A")
                        nc.tensor.matmul(
                            sc_ps[:],
                            qT[:, qt * 128:(qt + 1) * 128],
                            kT[:, sl],
                            start=True,
                            stop=True,
                        )
                        nc.vector.scalar_tensor_tensor(
                            out=masked_t[:, sl],
                            in0=sc_ps[:],
                            scalar=C_SHIFT,
                            in1=mask_t[:, sl],
                            op0=ALU.add,
                            op1=ALU.mult,
                        )
                    masked_g.append(masked_t)

                probsT = sb.tile([128, NT, 512], bf16, tag="pT")
                for kd in range(NT // 2):  # k-tile duos
                    tr_ps = pB.tile([128, 1024], f32, tag="B")
                    for j in range(2):
                        kt = kd * 2 + j
                        for qq in range(4):
                            nc.tensor.transpose(
                                tr_ps[:, j * 512 + qq * 128:j * 512 + (qq + 1) * 128],
                                masked_g[qq][:, kt * 128:(kt + 1) * 128],
                                id_r,
                            )
                    nc.scalar.activation(
                        probsT[:, kd * 2:kd * 2 + 2, :].rearrange("p a b -> p (a b)"),
                        tr_ps[:],
                        ACT_EXP,
                        bias=exp_bias[:],
                        scale=0.125,
                    )

                pv_ps = pPV.tile([H + 1, 512], f32, tag="pv")
                for kt in range(NT):
                    nc.tensor.matmul(
                        pv_ps[:],
                        v_aug[:, kt, :],
                        probsT[:, kt, :],
                        start=(kt == 0),
                        stop=(kt == NT - 1),
                    )
                oT_sb = sb.tile([H + 1, 512], f32, tag="oT")
                nc.scalar.copy(oT_sb[:], pv_ps[:])

                for qq in range(4):
                    qt = g * 4 + qq
                    o2_ps = pA.tile([128, H + 1], f32, tag="A")
                    nc.tensor.transpose(
                        o2_ps[:],
                        oT_sb[:, qq * 128:(qq + 1) * 128],
                        id_r[:H + 1, :H + 1],
                    )
                    rcp = sb.tile([128, 1], f32, tag="rcp")
                    nc.vector.reciprocal(rcp[:], o2_ps[:, H:H + 1])
                    out_sb = sb.tile([128, H], f32, tag="osb")
                    nc.vector.tensor_scalar_mul(
                        out_sb[:], o2_ps[:, 0:H], rcp[:]
                    )
                    nc.sync.dma_start(
                        out_d.ap()[qt * 128:(qt + 1) * 128, :], out_sb[:]
                    )

    nc.compile()
    return nc


_NC_CACHE = None


def _get_nc():
    global _NC_CACHE
    if _NC_CACHE is None:
        _NC_CACHE = build()
    return _NC_CACHE


def run(inputs, trace=False, trace_cores=None):
    nc = _get_nc()
    x = np.ascontiguousarray(np.asarray(inputs["input"], dtype=np.float32))
    m = np.ascontiguousarray(np.asarray(inputs["mask"], dtype=np.int32))
    shared = {
        n: np.ascontiguousarray(np.asarray(inputs[n], dtype=np.float32))
        for n in ("W_q", "b_q", "W_k", "b_k", "W_v", "b_v")
    }
    in_maps = [{"input": x[i], "mask": m[i], **shared} for i in range(B)]
    res = run_bass_kernel_spmd(
        nc,
        in_maps,
        core_ids=list(range(B)),
        trace=trace,
        trace_cores=trace_cores,
    )
    out = np.stack([res.results[i]["out"] for i in range(B)])
    return out, res


def kernel(**inputs) -> np.ndarray:
    out, _ = run(inputs, trace=False)
    return out
